# revision 18
# baseline (speedup 1.0000x reference)
"""Trainium2 Bass kernel for nn_Encoder_55490977464569 (binary-tree GRU encoder).

Strategy
--------
Data-parallel over batch: B=16 -> 2 batch elements per NeuronCore (8 cores),
zero collectives. Each core processes its whole tree (32767 nodes) leaves->root
entirely on-chip: all hidden states live in SBUF (bf16), only `targets` is
streamed in (host pre-transposed to feature-major so no on-device transposes).

v2 engine assignment (vs v1 which was ACT+DVE heavy):
  - r/z gates use the Sigmoid LUT directly (sigmoid_and_others table also
    holds tanh, so no table switches).  One ACT instruction covers the
    3T-wide [r_lo|r_hi|z] PSUM block.
  - s = rl*hl + rr*hr is materialized by a strided pair-add on DVE so the
    W_hn matmul is ONE 512-col pass instead of two (PE: 8 passes/parent).
  - child_sum runs on the otherwise idle GpSimd (Pool) engine.
  - blend ops are plain tensor_tensor (2x DVE mode); no scalar_tensor_tensor
    (which has no 2x microcode) on the hot path.

Layout: feature-major [128 features (partitions), node*batch rows (free)].
  - xi GEMMs:  K=32  (lhsT = W_i*.T [32,128] bf16, rhs = x feature-major)
  - hidden GEMMs: K=128 (lhsT = W_h*.T bf16)
  - PSUM accumulates xi + hidden GEMMs; ACT applies sigmoid/tanh from PSUM.
"""

import sys

if "/opt/trn_rl_repo" not in sys.path:
    sys.path.insert(0, "/opt/trn_rl_repo")
if "/opt/trn_rl_repo/concourse" not in sys.path:
    sys.path.insert(0, "/opt/trn_rl_repo/concourse")

import numpy as np
import ml_dtypes

from concourse import bass, mybir, tile, bacc
from concourse import bass_utils

BF16NP = ml_dtypes.bfloat16
F32 = mybir.dt.float32
BF16 = mybir.dt.bfloat16

N_CORES = 8
DEPTH = 15
HID = 128
IN_DIM = 32
OUT_DIM = 64
BATCH = 16
B_LOCAL = BATCH // N_CORES

T_TILE = 512      # parent rows per tile
H_CHUNK = 1024    # hidden-state chunk width (columns) per SBUF tile

POOL_CS = True    # child_sum on GpSimd (Pool) engine

ADD = mybir.AluOpType.add
SUB = mybir.AluOpType.subtract
MULT = mybir.AluOpType.mult
TANH = mybir.ActivationFunctionType.Tanh
SIGM = mybir.ActivationFunctionType.Sigmoid


def _level_rows(depth, b_local):
    return [2**l * b_local for l in range(depth)]


def _zoff(l, b_local):
    # column offset of level l in xz (heap order: nodes 0..N-1)
    return (2**l - 1) * b_local


def _roff(l, b_local):
    # column offset of level l in xr (levels 0..depth-2, each block 2*R_l wide)
    return (2**l - 1) * 2 * b_local


def build_program(depth=DEPTH, b_local=B_LOCAL, with_mask=False, with_bias=False):
    """Build the Bass program (same SPMD program for every core)."""
    nc = bacc.Bacc("TRN2", target_bir_lowering=False, debug=False,
                   num_devices=1)
    R = _level_rows(depth, b_local)
    total_z = sum(R)
    total_r = sum(2 * R[l] for l in range(depth - 1)) if depth > 1 else 0

    xz_d = nc.dram_tensor("xz", [IN_DIM, total_z], BF16, kind="ExternalInput")
    xr_d = None
    if total_r:
        xr_d = nc.dram_tensor("xr", [IN_DIM, total_r], BF16, kind="ExternalInput")
    # packed x for full-size tiles: 4x 32-row strips -> one [128, T] DMA
    pack_levels = [l for l in range(depth - 1) if R[l] >= T_TILE]
    pack_off = {}
    off = 0
    for l in pack_levels:
        pack_off[l] = off
        off += R[l]
    xpack_d = None
    if pack_levels:
        xpack_d = nc.dram_tensor("xpack", [128, off], BF16, kind="ExternalInput")
    leaf_pairs = (R[depth - 1] // T_TILE) // 2 if R[depth - 1] >= 2 * T_TILE else 0
    xleaf_d = None
    if leaf_pairs:
        xleaf_d = nc.dram_tensor("xleaf", [128, leaf_pairs * T_TILE], BF16,
                                 kind="ExternalInput")
    # all bf16 weights in one packed array -> a single startup DMA
    # columns: [w_hr | w_hz | w_hn | wx strips | wl strips | w_ir | w_iz | w_in]
    wcat_d = nc.dram_tensor("wcat", [128, 8 * HID], BF16, kind="ExternalInput")
    w_out_d = nc.dram_tensor("w_out", [HID, 2 * OUT_DIM], F32, kind="ExternalInput")
    out_d = nc.dram_tensor("out", [HID, b_local], F32, kind="ExternalOutput")
    if with_bias:
        # per-partition bias columns: [b_r | b_z | b_n | b_w | b_nl | b_out]
        bias_d = nc.dram_tensor("biases", [HID, 6], F32, kind="ExternalInput")
    if with_mask:
        mask_d = nc.dram_tensor("mask_bc", [HID, total_z], BF16, kind="ExternalInput")

    leaf = depth - 1

    from contextlib import ExitStack
    with tile.TileContext(nc) as tc, ExitStack() as stack:
        consts = stack.enter_context(tc.tile_pool(name="consts", bufs=1))
        hpool = stack.enter_context(tc.tile_pool(name="hpool", bufs=1))
        xpool = stack.enter_context(tc.tile_pool(name="xpool", bufs=6))
        apool = stack.enter_context(tc.tile_pool(name="apool", bufs=3))
        tpool = stack.enter_context(tc.tile_pool(name="tpool", bufs=4))
        cpool = stack.enter_context(tc.tile_pool(name="cpool", bufs=4))
        pspool = stack.enter_context(tc.tile_pool(name="pspool", bufs=2,
                                                  space="PSUM"))
        opool = stack.enter_context(tc.tile_pool(name="opool", bufs=1))

        wcat_sb = consts.tile([128, 8 * HID], BF16, name="wcat_sb", tag="wcat_sb")
        nc.sync.dma_start(out=wcat_sb, in_=wcat_d.ap())
        w_hr = wcat_sb[:, 0 * HID:1 * HID]
        w_hz = wcat_sb[:, 1 * HID:2 * HID]
        w_hn = wcat_sb[:, 2 * HID:3 * HID]
        wx_sb = wcat_sb[:, 3 * HID:4 * HID]
        wl_sb = wcat_sb[:, 4 * HID:5 * HID]
        w_ir = wcat_sb[0:IN_DIM, 5 * HID:6 * HID]
        w_iz = wcat_sb[0:IN_DIM, 6 * HID:7 * HID]
        w_in = wcat_sb[0:IN_DIM, 7 * HID:8 * HID]
        w_out = consts.tile([HID, 2 * OUT_DIM], F32, name="w_out_sb", tag="w_out_sb")
        nc.sync.dma_start(out=w_out, in_=w_out_d.ap())
        # preload x for all small (unpacked) levels: removes per-level DMAs
        # from the serial top-of-tree tail
        smalls = [l for l in range(depth - 1) if R[l] < T_TILE]
        xzs = xrs = None
        if smalls:
            zW = _zoff(smalls[-1] + 1, b_local)
            rW = _roff(smalls[-1] + 1, b_local)
            if zW:
                xzs = consts.tile([IN_DIM, zW], BF16, name="xzs", tag="xzs")
                nc.sync.dma_start(out=xzs, in_=xz_d.ap()[:, 0:zW])
            if rW:
                xrs = consts.tile([IN_DIM, rW], BF16, name="xrs", tag="xrs")
                nc.sync.dma_start(out=xrs, in_=xr_d.ap()[:, 0:rW])
        if with_bias:
            bias_sb = consts.tile([HID, 6], F32, name="bias_sb", tag="bias_sb")
            nc.sync.dma_start(out=bias_sb, in_=bias_d.ap())
            b_r, b_z, b_n = bias_sb[:, 0:1], bias_sb[:, 1:2], bias_sb[:, 2:3]
            b_w, b_nl, b_out = bias_sb[:, 3:4], bias_sb[:, 4:5], bias_sb[:, 5:6]

        # hidden-state tiles: h[l][c] is chunk c of level l (bf16)
        h_tiles = []
        for l in range(depth):
            cw = min(H_CHUNK, R[l])
            n_chunks = (R[l] + cw - 1) // cw
            h_tiles.append([
                hpool.tile([HID, cw], BF16, name=f"h_{l}_{c}", tag=f"h_{l}_{c}")
                for c in range(n_chunks)
            ])

        def mask_mul_inplace(view, lvl, col0, width):
            m_sb = xpool.tile([HID, width], BF16, name="m_sb", tag="m_sb")
            nc.sync.dma_start(
                out=m_sb, in_=mask_d.ap()[:, _zoff(lvl, b_local) + col0:
                                          _zoff(lvl, b_local) + col0 + width])
            nc.vector.tensor_mul(view, view, m_sb)

        def pair_add(eng, out_sb, in_view, width):
            """out[:, i] = in[:, 2i] + in[:, 2i+1] over `width` output cols."""
            i3 = in_view.rearrange("p (g f) -> p g f", f=4)
            o3 = out_sb.rearrange("p (g f) -> p g f", f=2)
            eng.tensor_add(o3, i3[:, :, 0:2], i3[:, :, 2:4])

        # PSUM accumulation groups are per 2KB bank (512 fp32 cols): every
        # start=True matmul must own its bank, so sub-512 blocks are placed
        # at bank-aligned column offsets inside the [HID, 3*T_TILE] tile.
        BANK = 512

        # ---------------- leaf level ----------------
        # h_leaf = sigmoid(-(W_iz x)) * tanh(W_in x); wl strips hold
        # [-w_iz | w_in | -w_iz | w_in] so PSUM gets [w | n] per tile.
        def leaf_tail(ps_w, ps_n, Tl, t0, wscale=1.0):
            """activations + h for one leaf tile, given its w|n psums done."""
            # separate full-tile outputs keep the h = w*n multiply in the
            # DVE 2x perf mode (shared/sliced operands fall back to 1x)
            w_sb = apool.tile([HID, Tl], BF16, name="w_leaf", tag="lw")
            n_sb = apool.tile([HID, Tl], BF16, name="n_leaf", tag="ln")
            if with_bias:
                nc.scalar.activation(w_sb, ps_w, SIGM, bias=b_w, scale=wscale)
                nc.scalar.activation(n_sb, ps_n, TANH, bias=b_nl)
            else:
                nc.scalar.activation(w_sb, ps_w, SIGM, scale=wscale)
                nc.scalar.activation(n_sb, ps_n, TANH)
            cidx, coff = t0 // H_CHUNK, t0 % H_CHUNK
            hview = h_tiles[leaf][cidx][:, coff:coff + Tl]
            nc.vector.tensor_mul(hview, w_sb, n_sb)
            if with_mask:
                mask_mul_inplace(hview, leaf, t0, Tl)

        Tl = min(T_TILE, R[leaf])
        n_leaf_tiles = R[leaf] // Tl
        for j in range(leaf_pairs):
            # two leaf tiles (2j, 2j+1) share one [128, T] packed x DMA.
            xp = xpool.tile([128, Tl], BF16, name="xp_leaf", tag="xp")
            nc.sync.dma_start(out=xp, in_=xleaf_d.ap()[:, j * Tl:(j + 1) * Tl])
            pss = []
            for u in range(2):
                ps = pspool.tile([HID, 3 * T_TILE], F32, name="ps_rz",
                                 tag="ps_rz")
                for i in range(2):
                    s = 2 * u + i
                    nc.tensor.matmul(ps[:, i * BANK:i * BANK + Tl],
                                     wl_sb[32 * s:32 * (s + 1)],
                                     xp[32 * s:32 * (s + 1)],
                                     start=True, stop=True,
                                     tile_position=(32 * s, 0))
                pss.append(ps)
            for u in range(2):
                leaf_tail(pss[u][:, 0:Tl], pss[u][:, BANK:BANK + Tl],
                          Tl, (2 * j + u) * Tl)

        for k in range(2 * leaf_pairs, n_leaf_tiles):
            t0 = k * Tl
            xz_sb = xpool.tile([IN_DIM, Tl], BF16, name="xz_sb", tag="xz")
            nc.sync.dma_start(out=xz_sb,
                              in_=xz_d.ap()[:, _zoff(leaf, b_local) + t0:
                                            _zoff(leaf, b_local) + t0 + Tl])
            ps = pspool.tile([HID, 3 * T_TILE], F32, name="ps_rz", tag="ps_rz")
            nc.tensor.matmul(ps[:, 0:Tl], w_iz, xz_sb,
                             start=True, stop=True)
            nc.tensor.matmul(ps[:, BANK:BANK + Tl], w_in, xz_sb,
                             start=True, stop=True)
            leaf_tail(ps[:, 0:Tl], ps[:, BANK:BANK + Tl], Tl, t0, wscale=-1.0)

        # ---------------- interior levels ----------------
        for l in range(depth - 2, -1, -1):
            T = min(T_TILE, R[l])
            C_child = min(H_CHUNK, R[l + 1])
            C_own = min(H_CHUNK, R[l])
            packed = l in pack_off

            # group flags: PSUM zeroing is per 2KB bank; the z block at
            # [2T:3T] shares bank 0 with r when 3T <= 512.
            z_fresh_bank = 3 * T > BANK
            pool_eng = nc.gpsimd if (POOL_CS and T == T_TILE) else nc.vector

            def make_cs(k, l=l, T=T, C_child=C_child):
                """child sum for tile k — issued one tile ahead so the Pool
                op is off the critical path."""
                t0 = k * T
                cidx, coff = (2 * t0) // C_child, (2 * t0) % C_child
                child = h_tiles[l + 1][cidx][:, coff:coff + 2 * T]
                cs_sb = cpool.tile([HID, T], BF16, name="cs_sb", tag="cs")
                pair_add(pool_eng, cs_sb, child, T)
                return cs_sb

            def stage_a(k, cs_sb, l=l, T=T, C_child=C_child, packed=packed):
                """xi + hr + hz matmuls, r/z activations, t, s. Returns state."""
                t0 = k * T
                cw = 2 * T
                cidx, coff = (2 * t0) // C_child, (2 * t0) % C_child
                child = h_tiles[l + 1][cidx][:, coff:coff + cw]
                st = {"child": child, "t0": t0, "cs_sb": cs_sb}

                ps_rz = pspool.tile([HID, 3 * T_TILE], F32, name="ps_rz",
                                    tag="ps_rz")
                ps_n = pspool.tile([HID, T_TILE], F32, name="ps_n",
                                   tag="ps_n")
                st["ps_rz"], st["ps_n"] = ps_rz, ps_n

                if packed:
                    # one [128, T] DMA; strips: [xi_r lo | xi_r hi | xi_z | xi_n]
                    xp = xpool.tile([128, T], BF16, name="xp_sb", tag="xp")
                    nc.sync.dma_start(out=xp,
                                      in_=xpack_d.ap()[:, pack_off[l] + t0:
                                                       pack_off[l] + t0 + T])
                    for s, dst in enumerate((ps_rz[:, 0:T], ps_rz[:, T:2 * T],
                                             ps_rz[:, 2 * T:3 * T])):
                        nc.tensor.matmul(dst, wx_sb[32 * s:32 * (s + 1)],
                                         xp[32 * s:32 * (s + 1)],
                                         start=True, stop=False,
                                         tile_position=(32 * s, 0))
                    nc.tensor.matmul(ps_n[:, 0:T], wx_sb[96:128], xp[96:128],
                                     start=True, stop=False,
                                     tile_position=(96, 0))
                else:
                    xr_sb = xrs[:, _roff(l, b_local) + 2 * t0:
                                _roff(l, b_local) + 2 * t0 + cw]
                    xz_sb = xzs[:, _zoff(l, b_local) + t0:
                                _zoff(l, b_local) + t0 + T]
                    nc.tensor.matmul(ps_rz[:, 0:cw], w_ir, xr_sb,
                                     start=True, stop=False)
                    nc.tensor.matmul(ps_rz[:, 2 * T:3 * T], w_iz, xz_sb,
                                     start=z_fresh_bank, stop=False)
                    nc.tensor.matmul(ps_n[:, 0:T], w_in, xz_sb,
                                     start=True, stop=False)

                # hr accumulate (child-row order), then hz over cs
                for i in range((cw + 511) // 512):
                    sl = slice(i * 512, min((i + 1) * 512, cw))
                    nc.tensor.matmul(ps_rz[:, sl], w_hr, child[:, sl],
                                     start=False, stop=z_fresh_bank)
                nc.tensor.matmul(ps_rz[:, 2 * T:3 * T], w_hz, cs_sb,
                                 start=False, stop=True)

                # rz_sb padded to 4T columns: pow2 row pitch keeps the DVE
                # 2x mode for the t and zd multiplies that read slices of it
                rz_sb = apool.tile([HID, 4 * T], BF16, name="rz_sb", tag="act")
                if not with_bias:
                    nc.scalar.activation(rz_sb[:, 0:3 * T], ps_rz[:, 0:3 * T],
                                         SIGM)
                else:
                    nc.scalar.activation(rz_sb[:, 0:cw], ps_rz[:, 0:cw], SIGM,
                                         bias=b_r)
                    nc.scalar.activation(rz_sb[:, cw:3 * T],
                                         ps_rz[:, cw:3 * T], SIGM, bias=b_z)
                st["rz_sb"] = rz_sb

                # t = r * child; s = pair-add(t)
                t_sb = tpool.tile([HID, cw], BF16, name="t_sb", tag="t")
                nc.vector.tensor_mul(t_sb, rz_sb[:, 0:cw], child)
                s_sb = tpool.tile([HID, T], BF16, name="s_sb", tag="s")
                pair_add(nc.vector, s_sb, t_sb, T)
                st["s_sb"] = s_sb
                return st

            def stage_b(k, st, l=l, T=T, C_own=C_own):
                """hn matmul, n activation, blend -> h."""
                t0 = st["t0"]
                cs_sb, rz_sb, ps_n = st["cs_sb"], st["rz_sb"], st["ps_n"]
                z_sb = rz_sb[:, 2 * T:3 * T]

                nc.tensor.matmul(ps_n[:, 0:T], w_hn, st["s_sb"],
                                 start=False, stop=True)
                n_sb = apool.tile([HID, T], BF16, name="n_sb", tag="act_s")
                if with_bias:
                    nc.scalar.activation(n_sb, ps_n[:, 0:T], TANH, bias=b_n)
                else:
                    nc.scalar.activation(n_sb, ps_n[:, 0:T], TANH)

                # h = n + z*(cs - n)
                d_sb = tpool.tile([HID, T], BF16, name="d_sb", tag="d")
                nc.vector.tensor_sub(d_sb, cs_sb, n_sb)
                zd_sb = tpool.tile([HID, T], BF16, name="zd_sb", tag="zd")
                nc.vector.tensor_mul(zd_sb, z_sb, d_sb)
                hidx, hoff = t0 // C_own, t0 % C_own
                hview = h_tiles[l][hidx][:, hoff:hoff + T]
                nc.vector.tensor_add(hview, zd_sb, n_sb)
                if with_mask:
                    mask_mul_inplace(hview, l, t0, T)

            # software-pipelined emission: stage A of tile k+1 interleaves
            # with stage B of tile k so the PE always has independent matmuls
            nt = R[l] // T
            prev_st = None
            cs_cur = make_cs(0)
            for k in range(nt):
                cs_nxt = make_cs(k + 1) if k + 1 < nt else None
                st = stage_a(k, cs_cur)
                cs_cur = cs_nxt
                if prev_st is not None:
                    stage_b(k - 1, prev_st)
                prev_st = st
            stage_b(nt - 1, prev_st)

        # ---------------- output head ----------------
        h0f = tpool.tile([HID, b_local], F32, name="h0f", tag="h0f")
        nc.vector.tensor_copy(h0f, h_tiles[0][0])
        ps_out = pspool.tile([HID, b_local], F32, name="ps_out", tag="ps_n")
        nc.tensor.matmul(ps_out, w_out, h0f, start=True, stop=True)
        out_sb = opool.tile([HID, b_local], F32, name="out_sb", tag="out_sb")
        if with_bias:
            nc.scalar.activation(out_sb, ps_out,
                                 mybir.ActivationFunctionType.Identity,
                                 bias=b_out)
        else:
            nc.scalar.copy(out_sb, ps_out)
        nc.sync.dma_start(out=out_d.ap(), in_=out_sb)

    nc.compile()
    return nc


def host_prep(inputs, depth=DEPTH, b_local=B_LOCAL, n_cores=N_CORES,
              with_mask=False, with_bias=False):
    """Build per-core input maps from the full problem inputs."""
    t = np.ascontiguousarray(np.asarray(inputs["targets"], np.float32))
    N = t.shape[0]
    assert N == 2**depth - 1 and t.shape[2] == IN_DIM
    R = _level_rows(depth, b_local)

    # feature-major, bf16: [32, N, B]
    xt = np.ascontiguousarray(t.transpose(2, 0, 1)).astype(BF16NP)

    def plain_t(w):
        return np.ascontiguousarray(np.asarray(w, np.float32).T).astype(BF16NP)

    w_ir = plain_t(inputs["W_ir"])
    w_iz = plain_t(inputs["W_iz"])
    w_in = plain_t(inputs["W_in"])
    w_izn = plain_t(-np.asarray(inputs["W_iz"], np.float32))
    w_hr = plain_t(inputs["W_hr"])
    w_hz = plain_t(inputs["W_hz"])
    w_hn = plain_t(inputs["W_hn"])
    w_out = np.ascontiguousarray(
        np.concatenate([np.asarray(inputs["W_mu"], np.float32),
                        np.asarray(inputs["W_lv"], np.float32)], axis=0).T)

    wcat = np.zeros((128, 8 * HID), BF16NP)
    wcat[:, 0 * HID:1 * HID] = w_hr
    wcat[:, 1 * HID:2 * HID] = w_hz
    wcat[:, 2 * HID:3 * HID] = w_hn
    for i, wsrc in enumerate((w_ir, w_ir, w_iz, w_in)):         # wx strips
        wcat[32 * i:32 * (i + 1), 3 * HID:4 * HID] = wsrc
    for i, wsrc in enumerate((w_izn, w_in, w_izn, w_in)):       # wl strips
        wcat[32 * i:32 * (i + 1), 4 * HID:5 * HID] = wsrc
    wcat[0:IN_DIM, 5 * HID:6 * HID] = w_ir
    wcat[0:IN_DIM, 6 * HID:7 * HID] = w_iz
    wcat[0:IN_DIM, 7 * HID:8 * HID] = w_in

    shared = dict(wcat=wcat, w_out=w_out)
    if with_bias:
        b = {k: np.asarray(inputs[k], np.float32) for k in
             ("b_ir", "b_hr", "b_iz", "b_hz", "b_in", "b_hn", "b_mu", "b_lv")}
        bias = np.zeros((HID, 6), np.float32)
        bias[:, 0] = b["b_ir"] + b["b_hr"]
        bias[:, 1] = b["b_iz"] + b["b_hz"]
        bias[:, 2] = b["b_in"] + b["b_hn"]
        # leaves: child_sum = s = 0, but b_hz / b_hn still apply
        bias[:, 3] = -(b["b_iz"] + b["b_hz"])
        bias[:, 4] = b["b_in"] + b["b_hn"]
        bias[:128, 5] = np.concatenate([b["b_mu"], b["b_lv"]])
        shared["biases"] = bias

    in_maps = []
    for c in range(n_cores):
        b0 = c * b_local
        xz = np.ascontiguousarray(
            xt[:, :, b0:b0 + b_local].reshape(IN_DIM, N * b_local))
        blocks = []
        for l in range(depth - 1):
            blk = xz[:, _zoff(l, b_local):_zoff(l, b_local) + R[l]]
            rep = np.repeat(blk.reshape(IN_DIM, -1, 1, 2), 2, axis=2)
            blocks.append(rep.reshape(IN_DIM, 2 * R[l]))
        m = dict(shared)
        m["xz"] = xz
        xr = np.concatenate(blocks, axis=1) if blocks else None
        if xr is not None:
            m["xr"] = np.ascontiguousarray(xr)
        # packed [128, T] blocks for the xi matmuls
        pack_levels = [l for l in range(depth - 1) if R[l] >= T_TILE]
        if pack_levels:
            pblocks = []
            for l in pack_levels:
                for k in range(R[l] // T_TILE):
                    t0 = k * T_TILE
                    rblk = xr[:, _roff(l, b_local) + 2 * t0:
                              _roff(l, b_local) + 2 * t0 + 2 * T_TILE]
                    zblk = xz[:, _zoff(l, b_local) + t0:
                              _zoff(l, b_local) + t0 + T_TILE]
                    pblocks.append(np.concatenate(
                        [rblk[:, :T_TILE], rblk[:, T_TILE:], zblk, zblk], axis=0))
            m["xpack"] = np.ascontiguousarray(np.concatenate(pblocks, axis=1))
        leaf = depth - 1
        leaf_pairs = (R[leaf] // T_TILE) // 2 if R[leaf] >= 2 * T_TILE else 0
        if leaf_pairs:
            lblocks = []
            for j in range(leaf_pairs):
                za = xz[:, _zoff(leaf, b_local) + 2 * j * T_TILE:
                        _zoff(leaf, b_local) + (2 * j + 1) * T_TILE]
                zb = xz[:, _zoff(leaf, b_local) + (2 * j + 1) * T_TILE:
                        _zoff(leaf, b_local) + (2 * j + 2) * T_TILE]
                lblocks.append(np.concatenate([za, za, zb, zb], axis=0))
            m["xleaf"] = np.ascontiguousarray(np.concatenate(lblocks, axis=1))
        if with_mask:
            mk = np.asarray(inputs["mask"], np.float32)[:, b0:b0 + b_local]
            m["mask_bc"] = np.ascontiguousarray(
                np.broadcast_to(mk.reshape(1, N * b_local),
                                (HID, N * b_local))).astype(BF16NP)
        in_maps.append(m)
    return in_maps


_PROGRAM_CACHE = {}


def _get_program(with_mask, with_bias):
    key = (with_mask, with_bias)
    if key not in _PROGRAM_CACHE:
        _PROGRAM_CACHE[key] = build_program(with_mask=with_mask,
                                            with_bias=with_bias)
    return _PROGRAM_CACHE[key]


def run_on_device(inputs, trace=False, **trace_kw):
    with_mask = not np.all(np.asarray(inputs["mask"]) == 1.0)
    with_bias = any(
        np.any(np.asarray(inputs[k]) != 0.0)
        for k in ("b_ir", "b_hr", "b_iz", "b_hz", "b_in", "b_hn", "b_mu", "b_lv"))
    nc = _get_program(with_mask, with_bias)
    in_maps = host_prep(inputs, with_mask=with_mask, with_bias=with_bias)
    res = bass_utils.run_bass_kernel_spmd(
        nc, in_maps, core_ids=list(range(N_CORES)), trace=trace, **trace_kw)
    mu = np.zeros((BATCH, OUT_DIM), np.float32)
    lv = np.zeros((BATCH, OUT_DIM), np.float32)
    for c in range(N_CORES):
        o = res.results[c]["out"]  # [128, b_local]
        mu[c * B_LOCAL:(c + 1) * B_LOCAL] = o[:OUT_DIM].T
        lv[c * B_LOCAL:(c + 1) * B_LOCAL] = o[OUT_DIM:].T
    return (mu, lv), res


def kernel(**inputs):
    (mu, lv), _ = run_on_device(inputs)
    return mu, lv


# revision 20
# speedup vs baseline: 1.0520x; 1.0520x over previous
"""Trainium2 Bass kernel for nn_Encoder_55490977464569 (binary-tree GRU encoder).

Strategy
--------
Data-parallel over batch: B=16 -> 2 batch elements per NeuronCore (8 cores),
zero collectives. Each core processes its whole tree (32767 nodes) leaves->root
entirely on-chip: all hidden states live in SBUF (bf16), only `targets` is
streamed in (host pre-transposed to feature-major so no on-device transposes).

v2 engine assignment (vs v1 which was ACT+DVE heavy):
  - r/z gates use the Sigmoid LUT directly (sigmoid_and_others table also
    holds tanh, so no table switches).  One ACT instruction covers the
    3T-wide [r_lo|r_hi|z] PSUM block.
  - s = rl*hl + rr*hr is materialized by a strided pair-add on DVE so the
    W_hn matmul is ONE 512-col pass instead of two (PE: 8 passes/parent).
  - child_sum runs on the otherwise idle GpSimd (Pool) engine.
  - blend ops are plain tensor_tensor (2x DVE mode); no scalar_tensor_tensor
    (which has no 2x microcode) on the hot path.

Layout: feature-major [128 features (partitions), node*batch rows (free)].
  - xi GEMMs:  K=32  (lhsT = W_i*.T [32,128] bf16, rhs = x feature-major)
  - hidden GEMMs: K=128 (lhsT = W_h*.T bf16)
  - PSUM accumulates xi + hidden GEMMs; ACT applies sigmoid/tanh from PSUM.
"""

import sys

if "/opt/trn_rl_repo" not in sys.path:
    sys.path.insert(0, "/opt/trn_rl_repo")
if "/opt/trn_rl_repo/concourse" not in sys.path:
    sys.path.insert(0, "/opt/trn_rl_repo/concourse")

import numpy as np
import ml_dtypes

from concourse import bass, mybir, tile, bacc
from concourse import bass_utils

BF16NP = ml_dtypes.bfloat16
F32 = mybir.dt.float32
BF16 = mybir.dt.bfloat16

N_CORES = 8
DEPTH = 15
HID = 128
IN_DIM = 32
OUT_DIM = 64
BATCH = 16
B_LOCAL = BATCH // N_CORES

T_TILE = 512      # parent rows per tile
H_CHUNK = 1024    # hidden-state chunk width (columns) per SBUF tile

POOL_CS = True    # child_sum on GpSimd (Pool) engine

ADD = mybir.AluOpType.add
SUB = mybir.AluOpType.subtract
MULT = mybir.AluOpType.mult
TANH = mybir.ActivationFunctionType.Tanh
SIGM = mybir.ActivationFunctionType.Sigmoid


def _level_rows(depth, b_local):
    return [2**l * b_local for l in range(depth)]


def _zoff(l, b_local):
    # column offset of level l in xz (heap order: nodes 0..N-1)
    return (2**l - 1) * b_local


def _roff(l, b_local):
    # column offset of level l in xr (levels 0..depth-2, each block 2*R_l wide)
    return (2**l - 1) * 2 * b_local


def build_program(depth=DEPTH, b_local=B_LOCAL, with_mask=False, with_bias=False):
    """Build the Bass program (same SPMD program for every core)."""
    nc = bacc.Bacc("TRN2", target_bir_lowering=False, debug=False,
                   num_devices=1)
    R = _level_rows(depth, b_local)
    total_z = sum(R)
    total_r = sum(2 * R[l] for l in range(depth - 1)) if depth > 1 else 0

    xz_d = nc.dram_tensor("xz", [IN_DIM, total_z], BF16, kind="ExternalInput")
    xr_d = None
    if total_r:
        xr_d = nc.dram_tensor("xr", [IN_DIM, total_r], BF16, kind="ExternalInput")
    # packed x for full-size tiles: 4x 32-row strips -> one [128, T] DMA
    pack_levels = [l for l in range(depth - 1) if R[l] >= T_TILE]
    pack_off = {}
    off = 0
    for l in pack_levels:
        pack_off[l] = off
        off += R[l]
    xpack_d = None
    if pack_levels:
        xpack_d = nc.dram_tensor("xpack", [128, off], BF16, kind="ExternalInput")
    leaf_pairs = (R[depth - 1] // T_TILE) // 2 if R[depth - 1] >= 2 * T_TILE else 0
    xleaf_d = None
    if leaf_pairs:
        xleaf_d = nc.dram_tensor("xleaf", [128, leaf_pairs * T_TILE], BF16,
                                 kind="ExternalInput")
    # all bf16 weights in one packed array -> a single startup DMA
    # columns: [w_hr | w_hz | w_hn | wx strips | wl strips | w_ir | w_iz | w_in]
    wcat_d = nc.dram_tensor("wcat", [128, 8 * HID], BF16, kind="ExternalInput")
    w_out_d = nc.dram_tensor("w_out", [HID, 2 * OUT_DIM], F32, kind="ExternalInput")
    out_d = nc.dram_tensor("out", [HID, b_local], F32, kind="ExternalOutput")
    if with_bias:
        # per-partition bias columns: [b_r | b_z | b_n | b_w | b_nl | b_out]
        bias_d = nc.dram_tensor("biases", [HID, 6], F32, kind="ExternalInput")
    if with_mask:
        mask_d = nc.dram_tensor("mask_bc", [HID, total_z], BF16, kind="ExternalInput")

    leaf = depth - 1

    from contextlib import ExitStack
    with tile.TileContext(nc) as tc, ExitStack() as stack:
        consts = stack.enter_context(tc.tile_pool(name="consts", bufs=1))
        hpool = stack.enter_context(tc.tile_pool(name="hpool", bufs=1))
        xpool = stack.enter_context(tc.tile_pool(name="xpool", bufs=6))
        apool = stack.enter_context(tc.tile_pool(name="apool", bufs=3))
        tpool = stack.enter_context(tc.tile_pool(name="tpool", bufs=4))
        cpool = stack.enter_context(tc.tile_pool(name="cpool", bufs=4))
        pspool = stack.enter_context(tc.tile_pool(name="pspool", bufs=2,
                                                  space="PSUM"))
        opool = stack.enter_context(tc.tile_pool(name="opool", bufs=1))

        wcat_sb = consts.tile([128, 8 * HID], BF16, name="wcat_sb", tag="wcat_sb")
        nc.sync.dma_start(out=wcat_sb, in_=wcat_d.ap())
        w_hr = wcat_sb[:, 0 * HID:1 * HID]
        w_hz = wcat_sb[:, 1 * HID:2 * HID]
        w_hn = wcat_sb[:, 2 * HID:3 * HID]
        wx_sb = wcat_sb[:, 3 * HID:4 * HID]
        wl_sb = wcat_sb[:, 4 * HID:5 * HID]
        w_ir = wcat_sb[0:IN_DIM, 5 * HID:6 * HID]
        w_iz = wcat_sb[0:IN_DIM, 6 * HID:7 * HID]
        w_in = wcat_sb[0:IN_DIM, 7 * HID:8 * HID]
        w_out = consts.tile([HID, 2 * OUT_DIM], F32, name="w_out_sb", tag="w_out_sb")
        nc.sync.dma_start(out=w_out, in_=w_out_d.ap())
        # preload x for all small (unpacked) levels: removes per-level DMAs
        # from the serial top-of-tree tail
        smalls = [l for l in range(depth - 1) if R[l] < T_TILE]
        xzs = xrs = None
        if smalls:
            zW = _zoff(smalls[-1] + 1, b_local)
            rW = _roff(smalls[-1] + 1, b_local)
            if zW:
                xzs = consts.tile([IN_DIM, zW], BF16, name="xzs", tag="xzs")
                nc.sync.dma_start(out=xzs, in_=xz_d.ap()[:, 0:zW])
            if rW:
                xrs = consts.tile([IN_DIM, rW], BF16, name="xrs", tag="xrs")
                nc.sync.dma_start(out=xrs, in_=xr_d.ap()[:, 0:rW])
        if with_bias:
            bias_sb = consts.tile([HID, 6], F32, name="bias_sb", tag="bias_sb")
            nc.sync.dma_start(out=bias_sb, in_=bias_d.ap())
            b_r, b_z, b_n = bias_sb[:, 0:1], bias_sb[:, 1:2], bias_sb[:, 2:3]
            b_w, b_nl, b_out = bias_sb[:, 3:4], bias_sb[:, 4:5], bias_sb[:, 5:6]

        # hidden-state tiles: h[l][c] is chunk c of level l (bf16)
        h_tiles = []
        for l in range(depth):
            cw = min(H_CHUNK, R[l])
            n_chunks = (R[l] + cw - 1) // cw
            h_tiles.append([
                hpool.tile([HID, cw], BF16, name=f"h_{l}_{c}", tag=f"h_{l}_{c}")
                for c in range(n_chunks)
            ])

        def mask_mul_inplace(view, lvl, col0, width):
            m_sb = xpool.tile([HID, width], BF16, name="m_sb", tag="m_sb")
            nc.sync.dma_start(
                out=m_sb, in_=mask_d.ap()[:, _zoff(lvl, b_local) + col0:
                                          _zoff(lvl, b_local) + col0 + width])
            nc.vector.tensor_mul(view, view, m_sb)

        def pair_add(eng, out_sb, in_view, width):
            """out[:, i] = in[:, 2i] + in[:, 2i+1] over `width` output cols."""
            i3 = in_view.rearrange("p (g f) -> p g f", f=4)
            o3 = out_sb.rearrange("p (g f) -> p g f", f=2)
            eng.tensor_add(o3, i3[:, :, 0:2], i3[:, :, 2:4])

        # PSUM accumulation groups are per 2KB bank (512 fp32 cols): every
        # start=True matmul must own its bank, so sub-512 blocks are placed
        # at bank-aligned column offsets inside the [HID, 3*T_TILE] tile.
        BANK = 512

        # ---------------- leaf level ----------------
        # h_leaf = sigmoid(-(W_iz x)) * tanh(W_in x); wl strips hold
        # [-w_iz | w_in | -w_iz | w_in] so PSUM gets [w | n] per tile.
        def leaf_tail(ps_w, ps_n, Tl, t0, wscale=1.0):
            """activations + h for one leaf tile, given its w|n psums done."""
            # separate full-tile outputs keep the h = w*n multiply in the
            # DVE 2x perf mode (shared/sliced operands fall back to 1x)
            w_sb = apool.tile([HID, Tl], BF16, name="w_leaf", tag="lw")
            n_sb = apool.tile([HID, Tl], BF16, name="n_leaf", tag="ln")
            if with_bias:
                nc.scalar.activation(w_sb, ps_w, SIGM, bias=b_w, scale=wscale)
                nc.scalar.activation(n_sb, ps_n, TANH, bias=b_nl)
            else:
                nc.scalar.activation(w_sb, ps_w, SIGM, scale=wscale)
                nc.scalar.activation(n_sb, ps_n, TANH)
            cidx, coff = t0 // H_CHUNK, t0 % H_CHUNK
            hview = h_tiles[leaf][cidx][:, coff:coff + Tl]
            nc.vector.tensor_mul(hview, w_sb, n_sb)
            if with_mask:
                mask_mul_inplace(hview, leaf, t0, Tl)

        Tl = min(T_TILE, R[leaf])
        n_leaf_tiles = R[leaf] // Tl
        for j in range(leaf_pairs):
            # two leaf tiles (2j, 2j+1) share one [128, T] packed x DMA.
            xp = xpool.tile([128, Tl], BF16, name="xp_leaf", tag="xp")
            nc.sync.dma_start(out=xp, in_=xleaf_d.ap()[:, j * Tl:(j + 1) * Tl])
            pss = []
            for u in range(2):
                ps = pspool.tile([HID, 3 * T_TILE], F32, name="ps_rz",
                                 tag="ps_rz")
                for i in range(2):
                    s = 2 * u + i
                    nc.tensor.matmul(ps[:, i * BANK:i * BANK + Tl],
                                     wl_sb[32 * s:32 * (s + 1)],
                                     xp[32 * s:32 * (s + 1)],
                                     start=True, stop=True,
                                     tile_position=(32 * s, 0))
                pss.append(ps)
            for u in range(2):
                leaf_tail(pss[u][:, 0:Tl], pss[u][:, BANK:BANK + Tl],
                          Tl, (2 * j + u) * Tl)

        for k in range(2 * leaf_pairs, n_leaf_tiles):
            t0 = k * Tl
            xz_sb = xpool.tile([IN_DIM, Tl], BF16, name="xz_sb", tag="xz")
            nc.sync.dma_start(out=xz_sb,
                              in_=xz_d.ap()[:, _zoff(leaf, b_local) + t0:
                                            _zoff(leaf, b_local) + t0 + Tl])
            ps = pspool.tile([HID, 3 * T_TILE], F32, name="ps_rz", tag="ps_rz")
            nc.tensor.matmul(ps[:, 0:Tl], w_iz, xz_sb,
                             start=True, stop=True)
            nc.tensor.matmul(ps[:, BANK:BANK + Tl], w_in, xz_sb,
                             start=True, stop=True)
            leaf_tail(ps[:, 0:Tl], ps[:, BANK:BANK + Tl], Tl, t0, wscale=-1.0)

        # ---------------- interior levels ----------------
        for l in range(depth - 2, -1, -1):
            T = min(T_TILE, R[l])
            C_child = min(H_CHUNK, R[l + 1])
            C_own = min(H_CHUNK, R[l])
            packed = l in pack_off

            # group flags: PSUM zeroing is per 2KB bank; the z block at
            # [2T:3T] shares bank 0 with r when 3T <= 512.
            z_fresh_bank = 3 * T > BANK
            pool_eng = nc.gpsimd if (POOL_CS and T == T_TILE) else nc.vector

            def make_cs(k, l=l, T=T, C_child=C_child):
                """child sum for tile k — issued one tile ahead so the Pool
                op is off the critical path."""
                t0 = k * T
                cidx, coff = (2 * t0) // C_child, (2 * t0) % C_child
                child = h_tiles[l + 1][cidx][:, coff:coff + 2 * T]
                cs_sb = cpool.tile([HID, T], BF16, name="cs_sb", tag="cs")
                pair_add(pool_eng, cs_sb, child, T)
                return cs_sb

            def stage_a(k, cs_sb, l=l, T=T, C_child=C_child, packed=packed):
                """xi + hr + hz matmuls, r/z activations, t, s. Returns state."""
                t0 = k * T
                cw = 2 * T
                cidx, coff = (2 * t0) // C_child, (2 * t0) % C_child
                child = h_tiles[l + 1][cidx][:, coff:coff + cw]
                st = {"child": child, "t0": t0, "cs_sb": cs_sb}

                ps_rz = pspool.tile([HID, 3 * T_TILE], F32, name="ps_rz",
                                    tag="ps_rz")
                ps_n = pspool.tile([HID, T_TILE], F32, name="ps_n",
                                   tag="ps_n")
                st["ps_rz"], st["ps_n"] = ps_rz, ps_n

                if packed:
                    # one [128, T] DMA; strips: [xi_r lo | xi_r hi | xi_z | xi_n]
                    xp = xpool.tile([128, T], BF16, name="xp_sb", tag="xp")
                    nc.sync.dma_start(out=xp,
                                      in_=xpack_d.ap()[:, pack_off[l] + t0:
                                                       pack_off[l] + t0 + T])
                if packed:
                    for s, dst in enumerate((ps_rz[:, 0:T],
                                             ps_rz[:, T:2 * T])):
                        nc.tensor.matmul(dst, wx_sb[32 * s:32 * (s + 1)],
                                         xp[32 * s:32 * (s + 1)],
                                         start=True, stop=False,
                                         tile_position=(32 * s, 0))
                    nc.tensor.matmul(ps_rz[:, 2 * T:3 * T], wx_sb[64:96],
                                     xp[64:96], start=True, stop=False,
                                     tile_position=(64, 0))
                else:
                    xr_sb = xrs[:, _roff(l, b_local) + 2 * t0:
                                _roff(l, b_local) + 2 * t0 + cw]
                    xz_sb = xzs[:, _zoff(l, b_local) + t0:
                                _zoff(l, b_local) + t0 + T]
                    nc.tensor.matmul(ps_rz[:, 0:cw], w_ir, xr_sb,
                                     start=True, stop=False)
                    nc.tensor.matmul(ps_rz[:, 2 * T:3 * T], w_iz, xz_sb,
                                     start=z_fresh_bank, stop=False)

                # hr accumulate (child-row order), then hz over cs
                for i in range((cw + 511) // 512):
                    sl = slice(i * 512, min((i + 1) * 512, cw))
                    nc.tensor.matmul(ps_rz[:, sl], w_hr, child[:, sl],
                                     start=False, stop=z_fresh_bank)
                nc.tensor.matmul(ps_rz[:, 2 * T:3 * T], w_hz, cs_sb,
                                 start=False, stop=True)
                # xi_n after hz: its ps_n buffer is released by tanh(k-2),
                # which lands well before this point in the ACT queue — the
                # PE never head-of-line blocks on it here
                if packed:
                    nc.tensor.matmul(ps_n[:, 0:T], wx_sb[96:128], xp[96:128],
                                     start=True, stop=False,
                                     tile_position=(96, 0))
                else:
                    nc.tensor.matmul(ps_n[:, 0:T], w_in, xz_sb,
                                     start=True, stop=False)

                # dedicated full-tile r/z outputs: sliced operands would drop
                # the downstream DVE multiplies from 2x to 1x mode
                r_sb = apool.tile([HID, cw], BF16, name="r_sb", tag="act")
                z_sb = apool.tile([HID, T], BF16, name="z_sb", tag="act_z")
                if not with_bias:
                    nc.scalar.activation(r_sb, ps_rz[:, 0:cw], SIGM)
                    nc.scalar.activation(z_sb, ps_rz[:, 2 * T:3 * T], SIGM)
                else:
                    nc.scalar.activation(r_sb, ps_rz[:, 0:cw], SIGM, bias=b_r)
                    nc.scalar.activation(z_sb, ps_rz[:, 2 * T:3 * T], SIGM,
                                         bias=b_z)
                st["z_sb"] = z_sb

                # t = r * child; s = pair-add(t)
                t_sb = tpool.tile([HID, cw], BF16, name="t_sb", tag="t")
                nc.vector.tensor_mul(t_sb, r_sb, child)
                s_sb = tpool.tile([HID, T], BF16, name="s_sb", tag="s")
                pair_add(nc.vector, s_sb, t_sb, T)
                st["s_sb"] = s_sb
                return st

            def stage_b(k, st, l=l, T=T, C_own=C_own):
                """hn matmul, n activation, blend -> h."""
                t0 = st["t0"]
                cs_sb, z_sb, ps_n = st["cs_sb"], st["z_sb"], st["ps_n"]

                nc.tensor.matmul(ps_n[:, 0:T], w_hn, st["s_sb"],
                                 start=False, stop=True)
                n_sb = apool.tile([HID, T], BF16, name="n_sb", tag="act_s")
                if with_bias:
                    nc.scalar.activation(n_sb, ps_n[:, 0:T], TANH, bias=b_n)
                else:
                    nc.scalar.activation(n_sb, ps_n[:, 0:T], TANH)

                # h = n + z*(cs - n)
                d_sb = tpool.tile([HID, T], BF16, name="d_sb", tag="d")
                nc.vector.tensor_sub(d_sb, cs_sb, n_sb)
                zd_sb = tpool.tile([HID, T], BF16, name="zd_sb", tag="zd")
                nc.vector.tensor_mul(zd_sb, z_sb, d_sb)
                hidx, hoff = t0 // C_own, t0 % C_own
                hview = h_tiles[l][hidx][:, hoff:hoff + T]
                nc.vector.tensor_add(hview, zd_sb, n_sb)
                if with_mask:
                    mask_mul_inplace(hview, l, t0, T)

            # software-pipelined emission: stage A of tile k+1 interleaves
            # with stage B of tile k so the PE always has independent matmuls
            nt = R[l] // T
            prev_st = None
            cs_cur = make_cs(0)
            for k in range(nt):
                cs_nxt = make_cs(k + 1) if k + 1 < nt else None
                st = stage_a(k, cs_cur)
                cs_cur = cs_nxt
                if prev_st is not None:
                    stage_b(k - 1, prev_st)
                prev_st = st
            stage_b(nt - 1, prev_st)

        # ---------------- output head ----------------
        h0f = tpool.tile([HID, b_local], F32, name="h0f", tag="h0f")
        nc.vector.tensor_copy(h0f, h_tiles[0][0])
        ps_out = pspool.tile([HID, b_local], F32, name="ps_out", tag="ps_n")
        nc.tensor.matmul(ps_out, w_out, h0f, start=True, stop=True)
        out_sb = opool.tile([HID, b_local], F32, name="out_sb", tag="out_sb")
        if with_bias:
            nc.scalar.activation(out_sb, ps_out,
                                 mybir.ActivationFunctionType.Identity,
                                 bias=b_out)
        else:
            nc.scalar.copy(out_sb, ps_out)
        nc.sync.dma_start(out=out_d.ap(), in_=out_sb)

    nc.compile()
    return nc


def host_prep(inputs, depth=DEPTH, b_local=B_LOCAL, n_cores=N_CORES,
              with_mask=False, with_bias=False):
    """Build per-core input maps from the full problem inputs."""
    t = np.ascontiguousarray(np.asarray(inputs["targets"], np.float32))
    N = t.shape[0]
    assert N == 2**depth - 1 and t.shape[2] == IN_DIM
    R = _level_rows(depth, b_local)

    # feature-major, bf16: [32, N, B]
    xt = np.ascontiguousarray(t.transpose(2, 0, 1)).astype(BF16NP)

    def plain_t(w):
        return np.ascontiguousarray(np.asarray(w, np.float32).T).astype(BF16NP)

    w_ir = plain_t(inputs["W_ir"])
    w_iz = plain_t(inputs["W_iz"])
    w_in = plain_t(inputs["W_in"])
    w_izn = plain_t(-np.asarray(inputs["W_iz"], np.float32))
    w_hr = plain_t(inputs["W_hr"])
    w_hz = plain_t(inputs["W_hz"])
    w_hn = plain_t(inputs["W_hn"])
    w_out = np.ascontiguousarray(
        np.concatenate([np.asarray(inputs["W_mu"], np.float32),
                        np.asarray(inputs["W_lv"], np.float32)], axis=0).T)

    wcat = np.zeros((128, 8 * HID), BF16NP)
    wcat[:, 0 * HID:1 * HID] = w_hr
    wcat[:, 1 * HID:2 * HID] = w_hz
    wcat[:, 2 * HID:3 * HID] = w_hn
    for i, wsrc in enumerate((w_ir, w_ir, w_iz, w_in)):         # wx strips
        wcat[32 * i:32 * (i + 1), 3 * HID:4 * HID] = wsrc
    for i, wsrc in enumerate((w_izn, w_in, w_izn, w_in)):       # wl strips
        wcat[32 * i:32 * (i + 1), 4 * HID:5 * HID] = wsrc
    wcat[0:IN_DIM, 5 * HID:6 * HID] = w_ir
    wcat[0:IN_DIM, 6 * HID:7 * HID] = w_iz
    wcat[0:IN_DIM, 7 * HID:8 * HID] = w_in

    shared = dict(wcat=wcat, w_out=w_out)
    if with_bias:
        b = {k: np.asarray(inputs[k], np.float32) for k in
             ("b_ir", "b_hr", "b_iz", "b_hz", "b_in", "b_hn", "b_mu", "b_lv")}
        bias = np.zeros((HID, 6), np.float32)
        bias[:, 0] = b["b_ir"] + b["b_hr"]
        bias[:, 1] = b["b_iz"] + b["b_hz"]
        bias[:, 2] = b["b_in"] + b["b_hn"]
        # leaves: child_sum = s = 0, but b_hz / b_hn still apply
        bias[:, 3] = -(b["b_iz"] + b["b_hz"])
        bias[:, 4] = b["b_in"] + b["b_hn"]
        bias[:128, 5] = np.concatenate([b["b_mu"], b["b_lv"]])
        shared["biases"] = bias

    in_maps = []
    for c in range(n_cores):
        b0 = c * b_local
        xz = np.ascontiguousarray(
            xt[:, :, b0:b0 + b_local].reshape(IN_DIM, N * b_local))
        blocks = []
        for l in range(depth - 1):
            blk = xz[:, _zoff(l, b_local):_zoff(l, b_local) + R[l]]
            rep = np.repeat(blk.reshape(IN_DIM, -1, 1, 2), 2, axis=2)
            blocks.append(rep.reshape(IN_DIM, 2 * R[l]))
        m = dict(shared)
        m["xz"] = xz
        xr = np.concatenate(blocks, axis=1) if blocks else None
        if xr is not None:
            m["xr"] = np.ascontiguousarray(xr)
        # packed [128, T] blocks for the xi matmuls
        pack_levels = [l for l in range(depth - 1) if R[l] >= T_TILE]
        if pack_levels:
            pblocks = []
            for l in pack_levels:
                for k in range(R[l] // T_TILE):
                    t0 = k * T_TILE
                    rblk = xr[:, _roff(l, b_local) + 2 * t0:
                              _roff(l, b_local) + 2 * t0 + 2 * T_TILE]
                    zblk = xz[:, _zoff(l, b_local) + t0:
                              _zoff(l, b_local) + t0 + T_TILE]
                    pblocks.append(np.concatenate(
                        [rblk[:, :T_TILE], rblk[:, T_TILE:], zblk, zblk], axis=0))
            m["xpack"] = np.ascontiguousarray(np.concatenate(pblocks, axis=1))
        leaf = depth - 1
        leaf_pairs = (R[leaf] // T_TILE) // 2 if R[leaf] >= 2 * T_TILE else 0
        if leaf_pairs:
            lblocks = []
            for j in range(leaf_pairs):
                za = xz[:, _zoff(leaf, b_local) + 2 * j * T_TILE:
                        _zoff(leaf, b_local) + (2 * j + 1) * T_TILE]
                zb = xz[:, _zoff(leaf, b_local) + (2 * j + 1) * T_TILE:
                        _zoff(leaf, b_local) + (2 * j + 2) * T_TILE]
                lblocks.append(np.concatenate([za, za, zb, zb], axis=0))
            m["xleaf"] = np.ascontiguousarray(np.concatenate(lblocks, axis=1))
        if with_mask:
            mk = np.asarray(inputs["mask"], np.float32)[:, b0:b0 + b_local]
            m["mask_bc"] = np.ascontiguousarray(
                np.broadcast_to(mk.reshape(1, N * b_local),
                                (HID, N * b_local))).astype(BF16NP)
        in_maps.append(m)
    return in_maps


_PROGRAM_CACHE = {}


def _get_program(with_mask, with_bias):
    key = (with_mask, with_bias)
    if key not in _PROGRAM_CACHE:
        _PROGRAM_CACHE[key] = build_program(with_mask=with_mask,
                                            with_bias=with_bias)
    return _PROGRAM_CACHE[key]


def run_on_device(inputs, trace=False, **trace_kw):
    with_mask = not np.all(np.asarray(inputs["mask"]) == 1.0)
    with_bias = any(
        np.any(np.asarray(inputs[k]) != 0.0)
        for k in ("b_ir", "b_hr", "b_iz", "b_hz", "b_in", "b_hn", "b_mu", "b_lv"))
    nc = _get_program(with_mask, with_bias)
    in_maps = host_prep(inputs, with_mask=with_mask, with_bias=with_bias)
    res = bass_utils.run_bass_kernel_spmd(
        nc, in_maps, core_ids=list(range(N_CORES)), trace=trace, **trace_kw)
    mu = np.zeros((BATCH, OUT_DIM), np.float32)
    lv = np.zeros((BATCH, OUT_DIM), np.float32)
    for c in range(N_CORES):
        o = res.results[c]["out"]  # [128, b_local]
        mu[c * B_LOCAL:(c + 1) * B_LOCAL] = o[:OUT_DIM].T
        lv[c * B_LOCAL:(c + 1) * B_LOCAL] = o[OUT_DIM:].T
    return (mu, lv), res


def kernel(**inputs):
    (mu, lv), _ = run_on_device(inputs)
    return mu, lv
